# revision 1
# baseline (speedup 1.0000x reference)
"""Deformable RoI pooling (deform_psroi_pooling, group_size=1) on 8 Trainium2
NeuronCores via Bass/Tile.

Strategy
--------
The reference computes, per roi r and output bin (ph, pw):

    out[r, c, ph, pw] = (1/max(cnt,1)) * sum_{valid samples s} bilinear(data[b_r, c], pos_s)

Every sample contributes 4 corner taps with weights independent of the
channel c.  Folding the bilinear weights, validity masking and the 1/cnt
normalisation together, each roi's output is a small matmul

    out[r, :, bin] = sum_{cells q} S_r[q, bin] * F[b_r, :, q]

with S_r a sparse per-roi weight matrix over the feature-map cells the roi
touches (computed on host in float32, exactly mirroring the reference
arithmetic), and F the feature map.

Device work per core (SPMD, one program, 8 cores):
  * feature map shipped channel-last as quad-cell rows [15200, 1024] f32
    (4 consecutive cells x 256 channels = 4KB per row),
  * indirect-DMA gathers of 128 quad-rows per instruction (one row per
    SBUF partition) pull exactly the cells its rois touch,
  * per 128-quad slot, 4 matmuls (lhsT = S slice [128,49], rhs = gathered
    channels [128,256]) accumulate into a [49, 256] PSUM tile per roi,
  * PSUM -> SBUF copy -> HBM out [49, NROI*256].

RoIs are dealt to cores of their image (cores 0-3 image 0, 4-7 image 1),
sorted by size and snake-dealt so all 8 cores execute an identical slot
profile; padding slots gather row 0 with all-zero S.
"""

import hashlib

import numpy as np

P = 7          # pooled size (== part size)
SPP = 4        # samples per part
SPATIAL_SCALE = np.float32(0.0625)
TRANS_STD = np.float32(0.1)
N_IMG, C_FEAT, H_FEAT, W_FEAT = 2, 256, 200, 304
QUAD = 4                      # cells per gathered row
NQROWS = H_FEAT * W_FEAT // QUAD          # 15200 quad rows per image
ROW_ELEMS = QUAD * C_FEAT                 # 1024 f32 per quad row
NBINS = P * P                             # 49
N_CORES = 8
SLOT_PAIRS = 128                          # quads per slot (one per partition)

_f32 = np.float32


def _host_tables(rois: np.ndarray, offset: np.ndarray):
    """Mirror the reference position math bit-exactly in float32 and build,
    per roi: the sorted list of quad-row ids it touches and the dense weight
    matrix S [nquads*4cells, 49] (weights already divided by max(cnt,1))."""
    R = rois.shape[0]
    rois = rois.astype(np.float32, copy=False)
    offset = offset.astype(np.float32, copy=False)

    b = rois[:, 0].astype(np.int32)
    roi_start_w = np.round(rois[:, 1]) * SPATIAL_SCALE - _f32(0.5)
    roi_start_h = np.round(rois[:, 2]) * SPATIAL_SCALE - _f32(0.5)
    roi_end_w = (np.round(rois[:, 3]) + _f32(1.0)) * SPATIAL_SCALE - _f32(0.5)
    roi_end_h = (np.round(rois[:, 4]) + _f32(1.0)) * SPATIAL_SCALE - _f32(0.5)
    roi_w = np.maximum(roi_end_w - roi_start_w, _f32(0.1))
    roi_h = np.maximum(roi_end_h - roi_start_h, _f32(0.1))
    bin_w = roi_w / _f32(P)
    bin_h = roi_h / _f32(P)
    sub_w = bin_w / _f32(SPP)
    sub_h = bin_h / _f32(SPP)

    ph = np.arange(P, dtype=np.float32)
    pw = np.arange(P, dtype=np.float32)
    # part_h == ph, part_w == pw for PART == P
    tx = offset[:, 0] * TRANS_STD                       # [R, P, P]
    ty = offset[:, 1] * TRANS_STD

    wstart = (pw[None, None, :] * bin_w[:, None, None]
              + roi_start_w[:, None, None] + tx * roi_w[:, None, None])
    hstart = (ph[None, :, None] * bin_h[:, None, None]
              + roi_start_h[:, None, None] + ty * roi_h[:, None, None])

    s = np.arange(SPP, dtype=np.float32)
    wpos = wstart[..., None, None] + s[None, None, None, None, :] * sub_w[:, None, None, None, None]
    hpos = hstart[..., None, None] + s[None, None, None, :, None] * sub_h[:, None, None, None, None]

    W = W_FEAT
    H = H_FEAT
    valid = ((wpos > _f32(-0.5)) & (wpos < _f32(W) - _f32(0.5))
             & (hpos > _f32(-0.5)) & (hpos < _f32(H) - _f32(0.5)))
    wc = np.clip(wpos, _f32(0.0), _f32(W - 1.0))
    hc = np.clip(hpos, _f32(0.0), _f32(H - 1.0))
    x0 = np.floor(wc)
    y0 = np.floor(hc)
    dx = wc - x0
    dy = hc - y0
    x0i = x0.astype(np.int32)
    y0i = y0.astype(np.int32)
    x1i = np.minimum(x0i + 1, W - 1)
    y1i = np.minimum(y0i + 1, H - 1)

    cnt = valid.sum(axis=(-1, -2)).astype(np.float32)           # [R, P, P]
    inv = _f32(1.0) / np.maximum(cnt, _f32(1.0))

    one = _f32(1.0)
    w00 = (one - dx) * (one - dy)
    w01 = dx * (one - dy)
    w10 = (one - dx) * dy
    w11 = dx * dy

    bins = np.broadcast_to(
        (np.arange(P)[:, None] * P + np.arange(P)[None, :])[None, :, :, None, None],
        valid.shape,
    )
    scale = np.broadcast_to(inv[:, :, :, None, None], valid.shape)

    per_roi = []
    for r in range(R):
        v = valid[r].ravel()
        if not v.any():
            per_roi.append((int(b[r]), np.zeros(1, np.int32),
                            np.zeros((1, QUAD, NBINS), np.float32)))
            continue
        shp = valid[r].shape
        bc = lambda a: np.broadcast_to(a, shp).ravel()[v]
        sc = bc(scale[r]).astype(np.float32)
        bn = bc(bins[r]).astype(np.int64)
        cy0 = bc(y0i[r]).astype(np.int64)
        cy1 = bc(y1i[r]).astype(np.int64)
        cx0 = bc(x0i[r]).astype(np.int64)
        cx1 = bc(x1i[r]).astype(np.int64)
        ws = [bc(w00[r]) * sc, bc(w01[r]) * sc,
              bc(w10[r]) * sc, bc(w11[r]) * sc]
        cells = [cy0 * W + cx0, cy0 * W + cx1, cy1 * W + cx0, cy1 * W + cx1]

        cell_all = np.concatenate(cells)
        w_all = np.concatenate(ws).astype(np.float64)
        bin_all = np.concatenate([bn] * 4)

        quads = np.unique(cell_all >> 2).astype(np.int32)       # sorted
        qpos = np.searchsorted(quads, cell_all >> 2)
        key = (qpos * QUAD + (cell_all & 3)) * NBINS + bin_all
        S = np.bincount(key, weights=w_all,
                        minlength=len(quads) * QUAD * NBINS)
        S = S.astype(np.float32).reshape(len(quads), QUAD, NBINS)
        per_roi.append((int(b[r]), quads, S))
    return per_roi


def _deal_to_cores(per_roi):
    """Assign rois to cores (cores 0-3 image 0, 4-7 image 1) snake-dealt by
    descending chunk count; build the shared slot profile."""
    img_rois = {0: [], 1: []}
    for rid, (img, quads, S) in enumerate(per_roi):
        nchunk = (len(quads) + SLOT_PAIRS - 1) // SLOT_PAIRS
        img_rois[img].append((nchunk, rid))
    core_rois = [[] for _ in range(N_CORES)]
    for img, lst in img_rois.items():
        lst.sort(reverse=True)
        cores = list(range(4 * img, 4 * img + 4))
        for i, item in enumerate(lst):
            k = i % 8
            c = cores[k] if k < 4 else cores[7 - k]
            core_rois[c].append(item)
    for c in range(N_CORES):
        core_rois[c].sort(reverse=True)          # descending chunk count
    nroi = max(1, max(len(cr) for cr in core_rois))
    profile = []
    for k in range(nroi):
        profile.append(max((cr[k][0] if k < len(cr) else 1)
                           for cr in core_rois))
    return core_rois, tuple(profile)


_PROGRAM_CACHE: dict = {}


def _build_program(profile):
    """One SPMD Tile program for all 8 cores, parameterised only by the slot
    profile (chunks per roi slot)."""
    key = profile
    if key in _PROGRAM_CACHE:
        return _PROGRAM_CACHE[key]

    from concourse import bass, mybir, bacc
    from concourse.tile import TileContext

    nroi = len(profile)
    nslot = sum(profile)

    nc = bacc.Bacc("TRN2", target_bir_lowering=False, debug=False,
                   num_devices=N_CORES)
    dataT = nc.declare_dram_parameter("dataT", [NQROWS, ROW_ELEMS],
                                      mybir.dt.float32, isOutput=False)
    offs = nc.declare_dram_parameter("offs", [128, nslot],
                                     mybir.dt.int32, isOutput=False)
    spack = nc.declare_dram_parameter("spack", [128, nslot * QUAD * NBINS],
                                      mybir.dt.float32, isOutput=False)
    out = nc.declare_dram_parameter("out", [NBINS, nroi * C_FEAT],
                                    mybir.dt.float32, isOutput=True)

    with TileContext(nc) as tc:
        with (
            tc.tile_pool(name="const", bufs=1) as cpool,
            tc.tile_pool(name="gt", bufs=6) as gpool,
            tc.tile_pool(name="ps", bufs=4, space="PSUM") as pspool,
            tc.tile_pool(name="ob", bufs=4) as opool,
        ):
            offs_t = cpool.tile([128, nslot], mybir.dt.int32)
            nc.sync.dma_start(out=offs_t[:], in_=offs[:])
            s_t = cpool.tile([128, nslot * QUAD * NBINS], mybir.dt.float32)
            # Load S in chunks so early matmuls can start sooner.
            scols = nslot * QUAD * NBINS
            nq = 8
            for q in range(nq):
                lo = q * scols // nq
                hi = (q + 1) * scols // nq
                nc.sync.dma_start(out=s_t[:, lo:hi], in_=spack[:, lo:hi])

            slot = 0
            for k in range(nroi):
                ps = pspool.tile([NBINS, C_FEAT], mybir.dt.float32)
                nch = profile[k]
                for j in range(nch):
                    gt = gpool.tile([128, ROW_ELEMS], mybir.dt.float32)
                    nc.gpsimd.indirect_dma_start(
                        out=gt[:],
                        out_offset=None,
                        in_=dataT[:],
                        in_offset=bass.IndirectOffsetOnAxis(
                            ap=offs_t[:, slot:slot + 1], axis=0),
                    )
                    for e in range(QUAD):
                        nc.tensor.matmul(
                            ps[:],
                            lhsT=s_t[:, (slot * QUAD + e) * NBINS:
                                     (slot * QUAD + e + 1) * NBINS],
                            rhs=gt[:, e * C_FEAT:(e + 1) * C_FEAT],
                            start=(j == 0 and e == 0),
                            stop=(j == nch - 1 and e == QUAD - 1),
                        )
                    slot += 1
                ob = opool.tile([NBINS, C_FEAT], mybir.dt.float32)
                nc.vector.tensor_copy(out=ob[:], in_=ps[:])
                nc.sync.dma_start(out=out[:, k * C_FEAT:(k + 1) * C_FEAT],
                                  in_=ob[:])
    nc.compile()
    _PROGRAM_CACHE[key] = nc
    return nc


def _core_inputs(per_roi, core_rois, profile, dataT_imgs):
    nroi = len(profile)
    nslot = sum(profile)
    base = np.cumsum([0] + list(profile))
    in_maps = []
    roi_of_slotk = []                      # per core: slot k -> roi id
    for c in range(N_CORES):
        img = 0 if c < 4 else 1
        offs = np.zeros((128, nslot), np.int32)
        spack = np.zeros((128, nslot * QUAD * NBINS), np.float32)
        rmap = [-1] * nroi
        for k, (nchunk, rid) in enumerate(core_rois[c]):
            rmap[k] = rid
            _, quads, S = per_roi[rid]
            npad = nchunk * SLOT_PAIRS
            qpad = np.zeros(npad, np.int32)
            qpad[:len(quads)] = quads
            Spad = np.zeros((npad, QUAD, NBINS), np.float32)
            Spad[:len(quads)] = S
            for j in range(nchunk):
                s0 = base[k] + j
                offs[:, s0] = qpad[j * 128:(j + 1) * 128]
                blk = Spad[j * 128:(j + 1) * 128]           # [128, 4, 49]
                spack[:, s0 * QUAD * NBINS:(s0 + 1) * QUAD * NBINS] = \
                    blk.reshape(128, QUAD * NBINS)
        in_maps.append({"dataT": dataT_imgs[img], "offs": offs,
                        "spack": spack})
        roi_of_slotk.append(rmap)
    return in_maps, roi_of_slotk


def kernel(data: np.ndarray, rois: np.ndarray, offset: np.ndarray) -> np.ndarray:
    from concourse.bass_utils import run_bass_kernel_spmd

    data = np.ascontiguousarray(data, dtype=np.float32)
    rois = np.asarray(rois, dtype=np.float32)
    offset = np.asarray(offset, dtype=np.float32)
    R = rois.shape[0]

    per_roi = _host_tables(rois, offset)
    core_rois, profile = _deal_to_cores(per_roi)
    nc = _build_program(profile)

    # channel-last quad-row layout per image: [15200, 1024] f32
    dataT_imgs = [
        np.ascontiguousarray(data[i].transpose(1, 2, 0)).reshape(NQROWS, ROW_ELEMS)
        for i in range(N_IMG)
    ]
    in_maps, roi_of_slotk = _core_inputs(per_roi, core_rois, profile, dataT_imgs)

    res = run_bass_kernel_spmd(nc, in_maps, list(range(N_CORES)), trace=False)

    out_full = np.zeros((R, C_FEAT, P, P), np.float32)
    nroi = len(profile)
    for c in range(N_CORES):
        o = np.asarray(res.results[c]["out"])          # [49, nroi*256]
        o = o.reshape(NBINS, nroi, C_FEAT).transpose(1, 2, 0)   # [nroi,256,49]
        for k, rid in enumerate(roi_of_slotk[c]):
            if rid >= 0:
                out_full[rid] = o[k].reshape(C_FEAT, P, P)
    return out_full



# revision 3
# speedup vs baseline: 1.7909x; 1.7909x over previous
"""Deformable RoI pooling (deform_psroi_pooling, group_size=1) on 8 Trainium2
NeuronCores via Bass/Tile.

Strategy (v2)
-------------
Per roi r and output bin (ph, pw) the reference computes a weighted sum of
feature-map cells; folding bilinear weights, validity masking and 1/cnt into
a per-roi sparse matrix S over touched cells, each roi's output is

    out[r, :, bin] = sum_{cells q} S_r[q, bin] * F[b_r, :, q]

Device layout:
  * both images shipped channel-last as quad-cell rows [2*15200, 1024] bf16
    (4 consecutive cells x 256 channels = 2KB per row),
  * per core, its rois' quad lists are packed back-to-back (at most 2 rois
    per 128-row chunk, padded only when a 3rd roi would enter a chunk;
    padding offsets are OOB so the gather skips them -> no wire bytes),
  * per chunk: one indirect-DMA gather of 128 quad rows, then 4 bf16
    matmuls (lhsT = S slice [128, 98] covering the chunk's <=2 rois in two
    49-bin parity blocks) into a [98, 256] fp32 PSUM tile,
  * PSUM -> SBUF (bf16) -> HBM, one DMA per group of G chunks,
  * host sums per-roi partials across chunks in fp32.

RoIs are globally balanced across all 8 cores by quad count (LPT greedy);
every core runs the identical program parameterised only by the chunk
count C.
"""

import numpy as np

P = 7          # pooled size (== part size)
SPP = 4        # samples per part
SPATIAL_SCALE = np.float32(0.0625)
TRANS_STD = np.float32(0.1)
N_IMG, C_FEAT, H_FEAT, W_FEAT = 2, 256, 200, 304
QUAD = 4                                  # cells per gathered row
NQROWS = H_FEAT * W_FEAT // QUAD          # 15200 quad rows per image
NROWS_ALL = N_IMG * NQROWS                # both images stacked
ROW_ELEMS = QUAD * C_FEAT                 # 1024 elems per quad row
NBINS = P * P                             # 49
MBLK = 2 * NBINS                          # 98: two parity blocks of bins
N_CORES = 8
CHUNK = 128                               # quad rows per gather chunk
OOB = 0x3FFFFFFF                          # padding offset, skipped by gather
GOUT = 4                                  # chunks per output DMA

_f32 = np.float32


def _host_tables(rois: np.ndarray, offset: np.ndarray):
    """Mirror the reference position math bit-exactly in float32 and build,
    per roi: the sorted list of global quad-row ids it touches and the dense
    weight matrix S [nquads, QUAD, NBINS] (weights already / max(cnt,1))."""
    R = rois.shape[0]
    rois = rois.astype(np.float32, copy=False)
    offset = offset.astype(np.float32, copy=False)

    b = rois[:, 0].astype(np.int32)
    roi_start_w = np.round(rois[:, 1]) * SPATIAL_SCALE - _f32(0.5)
    roi_start_h = np.round(rois[:, 2]) * SPATIAL_SCALE - _f32(0.5)
    roi_end_w = (np.round(rois[:, 3]) + _f32(1.0)) * SPATIAL_SCALE - _f32(0.5)
    roi_end_h = (np.round(rois[:, 4]) + _f32(1.0)) * SPATIAL_SCALE - _f32(0.5)
    roi_w = np.maximum(roi_end_w - roi_start_w, _f32(0.1))
    roi_h = np.maximum(roi_end_h - roi_start_h, _f32(0.1))
    bin_w = roi_w / _f32(P)
    bin_h = roi_h / _f32(P)
    sub_w = bin_w / _f32(SPP)
    sub_h = bin_h / _f32(SPP)

    ph = np.arange(P, dtype=np.float32)
    pw = np.arange(P, dtype=np.float32)
    # part_h == ph, part_w == pw for PART == P
    tx = offset[:, 0] * TRANS_STD                       # [R, P, P]
    ty = offset[:, 1] * TRANS_STD

    wstart = (pw[None, None, :] * bin_w[:, None, None]
              + roi_start_w[:, None, None] + tx * roi_w[:, None, None])
    hstart = (ph[None, :, None] * bin_h[:, None, None]
              + roi_start_h[:, None, None] + ty * roi_h[:, None, None])

    s = np.arange(SPP, dtype=np.float32)
    wpos = wstart[..., None, None] + s[None, None, None, None, :] * sub_w[:, None, None, None, None]
    hpos = hstart[..., None, None] + s[None, None, None, :, None] * sub_h[:, None, None, None, None]

    W = W_FEAT
    H = H_FEAT
    valid = ((wpos > _f32(-0.5)) & (wpos < _f32(W) - _f32(0.5))
             & (hpos > _f32(-0.5)) & (hpos < _f32(H) - _f32(0.5)))
    wc = np.clip(wpos, _f32(0.0), _f32(W - 1.0))
    hc = np.clip(hpos, _f32(0.0), _f32(H - 1.0))
    x0 = np.floor(wc)
    y0 = np.floor(hc)
    dx = wc - x0
    dy = hc - y0
    x0i = x0.astype(np.int32)
    y0i = y0.astype(np.int32)
    x1i = np.minimum(x0i + 1, W - 1)
    y1i = np.minimum(y0i + 1, H - 1)

    cnt = valid.sum(axis=(-1, -2)).astype(np.float32)           # [R, P, P]
    inv = _f32(1.0) / np.maximum(cnt, _f32(1.0))

    one = _f32(1.0)
    w00 = (one - dx) * (one - dy)
    w01 = dx * (one - dy)
    w10 = (one - dx) * dy
    w11 = dx * dy

    bins = np.broadcast_to(
        (np.arange(P)[:, None] * P + np.arange(P)[None, :])[None, :, :, None, None],
        valid.shape,
    )
    scale = np.broadcast_to(inv[:, :, :, None, None], valid.shape)

    per_roi = []
    for r in range(R):
        v = valid[r].ravel()
        if not v.any():
            per_roi.append((np.zeros(0, np.int32),
                            np.zeros((0, QUAD, NBINS), np.float32)))
            continue
        shp = valid[r].shape
        bc = lambda a: np.broadcast_to(a, shp).ravel()[v]
        sc = bc(scale[r]).astype(np.float32)
        bn = bc(bins[r]).astype(np.int64)
        cy0 = bc(y0i[r]).astype(np.int64)
        cy1 = bc(y1i[r]).astype(np.int64)
        cx0 = bc(x0i[r]).astype(np.int64)
        cx1 = bc(x1i[r]).astype(np.int64)
        ws = [bc(w00[r]) * sc, bc(w01[r]) * sc,
              bc(w10[r]) * sc, bc(w11[r]) * sc]
        cells = [cy0 * W + cx0, cy0 * W + cx1, cy1 * W + cx0, cy1 * W + cx1]

        cell_all = np.concatenate(cells)
        w_all = np.concatenate(ws).astype(np.float64)
        bin_all = np.concatenate([bn] * 4)

        quads = np.unique(cell_all >> 2).astype(np.int32)       # sorted
        qpos = np.searchsorted(quads, cell_all >> 2)
        key = (qpos * QUAD + (cell_all & 3)) * NBINS + bin_all
        S = np.bincount(key, weights=w_all,
                        minlength=len(quads) * QUAD * NBINS)
        S = S.astype(np.float32).reshape(len(quads), QUAD, NBINS)
        per_roi.append((quads + np.int32(b[r]) * np.int32(NQROWS), S))
    return per_roi


def _balance(per_roi):
    """LPT greedy: assign rois to 8 cores balancing total quad count."""
    order = sorted(range(len(per_roi)),
                   key=lambda r: -len(per_roi[r][0]))
    loads = [0] * N_CORES
    core_rois = [[] for _ in range(N_CORES)]
    for rid in order:
        c = min(range(N_CORES), key=lambda i: loads[i])
        core_rois[c].append(rid)
        loads[c] += len(per_roi[rid][0])
    return core_rois


def _pack_core(rids, per_roi):
    """Pack rois' quad rows back-to-back; pad to the next chunk boundary
    only when a 3rd roi would enter a chunk. Returns list of
    (rid, start_row) and total rows used."""
    pos = 0
    placements = []
    chunk_rois = {}          # chunk idx -> count of rois touching it
    for rid in rids:
        q = len(per_roi[rid][0])
        if q == 0:
            placements.append((rid, pos))
            continue
        c0 = pos // CHUNK
        if chunk_rois.get(c0, 0) >= 2 and pos % CHUNK != 0:
            pos = (c0 + 1) * CHUNK
            c0 = pos // CHUNK
        placements.append((rid, pos))
        for c in range(c0, (pos + q - 1) // CHUNK + 1):
            chunk_rois[c] = chunk_rois.get(c, 0) + 1
        pos += q
    return placements, pos


_PROGRAM_CACHE: dict = {}


def _build_program(C):
    """One SPMD Tile program for all 8 cores, parameterised by chunk count."""
    if C in _PROGRAM_CACHE:
        return _PROGRAM_CACHE[C]

    from concourse import bass, mybir, bacc
    from concourse.tile import TileContext

    nc = bacc.Bacc("TRN2", target_bir_lowering=False, debug=False,
                   num_devices=N_CORES)
    dataT = nc.declare_dram_parameter("dataT", [NROWS_ALL, ROW_ELEMS],
                                      mybir.dt.bfloat16, isOutput=False)
    offs = nc.declare_dram_parameter("offs", [CHUNK, C],
                                     mybir.dt.int32, isOutput=False)
    spack = nc.declare_dram_parameter("spack", [CHUNK, C * QUAD * MBLK],
                                      mybir.dt.bfloat16, isOutput=False)
    out = nc.declare_dram_parameter("out", [MBLK, C * C_FEAT],
                                    mybir.dt.bfloat16, isOutput=True)

    with TileContext(nc) as tc:
        with (
            tc.tile_pool(name="const", bufs=1) as cpool,
            tc.tile_pool(name="gt", bufs=6) as gpool,
            tc.tile_pool(name="ps", bufs=6, space="PSUM") as pspool,
            tc.tile_pool(name="ob", bufs=3) as opool,
        ):
            offs_t = cpool.tile([CHUNK, C], mybir.dt.int32)
            nc.sync.dma_start(out=offs_t[:], in_=offs[:])
            s_t = cpool.tile([CHUNK, C * QUAD * MBLK], mybir.dt.bfloat16)
            # Load S per chunk-group so early matmuls can start sooner.
            scw = QUAD * MBLK
            for g in range(0, C, GOUT):
                hi = min(g + GOUT, C)
                nc.sync.dma_start(out=s_t[:, g * scw:hi * scw],
                                  in_=spack[:, g * scw:hi * scw])

            ob = None
            for k in range(C):
                gt = gpool.tile([CHUNK, ROW_ELEMS], mybir.dt.bfloat16)
                nc.gpsimd.indirect_dma_start(
                    out=gt[:],
                    out_offset=None,
                    in_=dataT[:],
                    in_offset=bass.IndirectOffsetOnAxis(
                        ap=offs_t[:, k:k + 1], axis=0),
                    bounds_check=NROWS_ALL - 1,
                    oob_is_err=False,
                )
                ps = pspool.tile([MBLK, C_FEAT], mybir.dt.float32)
                for e in range(QUAD):
                    nc.tensor.matmul(
                        ps[:],
                        lhsT=s_t[:, (k * QUAD + e) * MBLK:
                                 (k * QUAD + e + 1) * MBLK],
                        rhs=gt[:, e * C_FEAT:(e + 1) * C_FEAT],
                        start=(e == 0),
                        stop=(e == QUAD - 1),
                    )
                if k % GOUT == 0:
                    gw = min(GOUT, C - k)
                    ob = opool.tile([MBLK, gw * C_FEAT], mybir.dt.bfloat16)
                j = k % GOUT
                if k % 2 == 0:
                    nc.vector.tensor_copy(
                        out=ob[:, j * C_FEAT:(j + 1) * C_FEAT], in_=ps[:])
                else:
                    nc.scalar.copy(
                        out=ob[:, j * C_FEAT:(j + 1) * C_FEAT], in_=ps[:])
                if j == gw - 1:
                    k0 = k - j
                    nc.sync.dma_start(
                        out=out[:, k0 * C_FEAT:(k + 1) * C_FEAT], in_=ob[:])
    nc.compile()
    _PROGRAM_CACHE[C] = nc
    return nc


def _to_bf16(x):
    import ml_dtypes
    return x.astype(ml_dtypes.bfloat16)


def _core_inputs(per_roi, core_rois, C, dataT_b):
    in_maps = []
    chunk_maps = []          # per core: list over chunks of [rid_b0, rid_b1]
    import ml_dtypes
    for c in range(N_CORES):
        placements, _ = _pack_core(core_rois[c], per_roi)
        offs = np.full((CHUNK, C), OOB, np.int32)
        spack = np.zeros((CHUNK, C * QUAD * MBLK), np.float32)
        cmap = [[-1, -1] for _ in range(C)]
        nplaced = 0
        for rid, start in placements:
            quads, S = per_roi[rid]
            q = len(quads)
            if q == 0:
                continue
            beta = nplaced % 2
            nplaced += 1
            rows = np.arange(start, start + q)
            ck = rows // CHUNK
            pp = rows % CHUNK
            offs[pp, ck] = quads
            # scatter S into spack: col = (ck*QUAD + e)*MBLK + beta*49 + bin
            for e in range(QUAD):
                cols = (ck * QUAD + e) * MBLK + beta * NBINS
                # S[:, e, :] -> spack[pp, cols:cols+49]
                spack[pp[:, None], cols[:, None] + np.arange(NBINS)[None, :]] = S[:, e, :]
            for kk in np.unique(ck):
                cmap[kk][beta] = rid
        in_maps.append({
            "dataT": dataT_b,
            "offs": offs,
            "spack": spack.astype(ml_dtypes.bfloat16),
        })
        chunk_maps.append(cmap)
    return in_maps, chunk_maps


def kernel(data: np.ndarray, rois: np.ndarray, offset: np.ndarray) -> np.ndarray:
    from concourse.bass_utils import run_bass_kernel_spmd

    data = np.ascontiguousarray(data, dtype=np.float32)
    rois = np.asarray(rois, dtype=np.float32)
    offset = np.asarray(offset, dtype=np.float32)
    R = rois.shape[0]

    per_roi = _host_tables(rois, offset)
    core_rois = _balance(per_roi)
    C = 0
    for c in range(N_CORES):
        _, rows = _pack_core(core_rois[c], per_roi)
        C = max(C, (rows + CHUNK - 1) // CHUNK)
    nc = _build_program(C)

    # channel-last quad-row layout, both images stacked: [30400, 1024] bf16
    dataT = np.concatenate([
        np.ascontiguousarray(data[i].transpose(1, 2, 0)).reshape(NQROWS, ROW_ELEMS)
        for i in range(N_IMG)
    ], axis=0)
    dataT_b = _to_bf16(dataT)
    in_maps, chunk_maps = _core_inputs(per_roi, core_rois, C, dataT_b)

    res = run_bass_kernel_spmd(nc, in_maps, list(range(N_CORES)), trace=False)

    out_full = np.zeros((R, C_FEAT, P, P), np.float32)
    for c in range(N_CORES):
        o = np.asarray(res.results[c]["out"]).astype(np.float32)  # [98, C*256]
        o = o.reshape(MBLK, C, C_FEAT).transpose(1, 0, 2)         # [C, 98, 256]
        for k in range(C):
            for beta in range(2):
                rid = chunk_maps[c][k][beta]
                if rid >= 0:
                    blk = o[k, beta * NBINS:(beta + 1) * NBINS]   # [49, 256]
                    out_full[rid] += blk.T.reshape(C_FEAT, P, P)
    return out_full


# revision 11
# speedup vs baseline: 1.9067x; 1.0647x over previous
"""Deformable RoI pooling (deform_psroi_pooling, group_size=1) on 8 Trainium2
NeuronCores via Bass/Tile.

Strategy (v2)
-------------
Per roi r and output bin (ph, pw) the reference computes a weighted sum of
feature-map cells; folding bilinear weights, validity masking and 1/cnt into
a per-roi sparse matrix S over touched cells, each roi's output is

    out[r, :, bin] = sum_{cells q} S_r[q, bin] * F[b_r, :, q]

Device layout:
  * both images shipped channel-last as quad-cell rows [2*15200+2, 1024]
    bf16 (4 consecutive cells x 256 channels = 2KB per row, 2 zero pad
    rows at the end),
  * each roi's sorted quad list is decomposed into runs of consecutive
    quads, then segments of <= 2 consecutive quads; one segment = one
    partition slot; the indirect gather fetches K=2 consecutive rows per
    offset (4KB per slot),
  * per core, its rois' segment lists are packed back-to-back (at most 2
    rois per 128-slot chunk, padded only when a 3rd roi would enter a
    chunk; padding offsets are OOB so the gather skips them),
  * per chunk: one indirect-DMA gather of 128 x 2 quad rows, then 8 bf16
    matmuls (lhsT = S slice [128, 98] covering the chunk's <=2 rois in two
    49-bin parity blocks) into a [98, 256] fp32 PSUM tile,
  * PSUM -> SBUF (bf16) -> HBM, one DMA per group of G chunks,
  * host sums per-roi partials across chunks in fp32.

RoIs are globally balanced across all 8 cores by segment count (LPT
greedy); every core runs the identical program parameterised only by the
chunk count C.
"""

import numpy as np

P = 7          # pooled size (== part size)
SPP = 4        # samples per part
SPATIAL_SCALE = np.float32(0.0625)
TRANS_STD = np.float32(0.1)
N_IMG, C_FEAT, H_FEAT, W_FEAT = 2, 256, 200, 304
QUAD = 4                                  # cells per quad row
KSEG = 2                                  # consecutive quad rows per segment
NQROWS = H_FEAT * W_FEAT // QUAD          # 15200 quad rows per image
NROWS_ALL = N_IMG * NQROWS                # both images stacked
ROW_ELEMS = QUAD * C_FEAT                 # 1024 elems per quad row
SEG_ELEMS = KSEG * ROW_ELEMS              # 2048 elems per gathered slot
SEG_CELLS = KSEG * QUAD                   # 8 cells per slot
NBINS = P * P                             # 49
MBLK = 2 * NBINS                          # 98: two parity blocks of bins
N_CORES = 8
CHUNK = 128                               # segment slots per gather chunk
OOB = 0x3FFFFFFF                          # padding offset, skipped by gather
GOUT = 4                                  # chunks per output DMA

_f32 = np.float32


def _host_tables(rois: np.ndarray, offset: np.ndarray):
    """Mirror the reference position math bit-exactly in float32 and build,
    per roi: the sorted list of global quad-row ids it touches and the dense
    weight matrix S [nquads, QUAD, NBINS] (weights already / max(cnt,1))."""
    R = rois.shape[0]
    rois = rois.astype(np.float32, copy=False)
    offset = offset.astype(np.float32, copy=False)

    b = rois[:, 0].astype(np.int32)
    roi_start_w = np.round(rois[:, 1]) * SPATIAL_SCALE - _f32(0.5)
    roi_start_h = np.round(rois[:, 2]) * SPATIAL_SCALE - _f32(0.5)
    roi_end_w = (np.round(rois[:, 3]) + _f32(1.0)) * SPATIAL_SCALE - _f32(0.5)
    roi_end_h = (np.round(rois[:, 4]) + _f32(1.0)) * SPATIAL_SCALE - _f32(0.5)
    roi_w = np.maximum(roi_end_w - roi_start_w, _f32(0.1))
    roi_h = np.maximum(roi_end_h - roi_start_h, _f32(0.1))
    bin_w = roi_w / _f32(P)
    bin_h = roi_h / _f32(P)
    sub_w = bin_w / _f32(SPP)
    sub_h = bin_h / _f32(SPP)

    ph = np.arange(P, dtype=np.float32)
    pw = np.arange(P, dtype=np.float32)
    # part_h == ph, part_w == pw for PART == P
    tx = offset[:, 0] * TRANS_STD                       # [R, P, P]
    ty = offset[:, 1] * TRANS_STD

    wstart = (pw[None, None, :] * bin_w[:, None, None]
              + roi_start_w[:, None, None] + tx * roi_w[:, None, None])
    hstart = (ph[None, :, None] * bin_h[:, None, None]
              + roi_start_h[:, None, None] + ty * roi_h[:, None, None])

    s = np.arange(SPP, dtype=np.float32)
    wpos = wstart[..., None, None] + s[None, None, None, None, :] * sub_w[:, None, None, None, None]
    hpos = hstart[..., None, None] + s[None, None, None, :, None] * sub_h[:, None, None, None, None]

    W = W_FEAT
    H = H_FEAT
    valid = ((wpos > _f32(-0.5)) & (wpos < _f32(W) - _f32(0.5))
             & (hpos > _f32(-0.5)) & (hpos < _f32(H) - _f32(0.5)))
    wc = np.clip(wpos, _f32(0.0), _f32(W - 1.0))
    hc = np.clip(hpos, _f32(0.0), _f32(H - 1.0))
    x0 = np.floor(wc)
    y0 = np.floor(hc)
    dx = wc - x0
    dy = hc - y0
    x0i = x0.astype(np.int32)
    y0i = y0.astype(np.int32)
    x1i = np.minimum(x0i + 1, W - 1)
    y1i = np.minimum(y0i + 1, H - 1)

    cnt = valid.sum(axis=(-1, -2)).astype(np.float32)           # [R, P, P]
    inv = _f32(1.0) / np.maximum(cnt, _f32(1.0))

    one = _f32(1.0)
    w00 = (one - dx) * (one - dy)
    w01 = dx * (one - dy)
    w10 = (one - dx) * dy
    w11 = dx * dy

    bins = np.broadcast_to(
        (np.arange(P)[:, None] * P + np.arange(P)[None, :])[None, :, :, None, None],
        valid.shape,
    )
    scale = np.broadcast_to(inv[:, :, :, None, None], valid.shape)

    per_roi = []
    for r in range(R):
        v = valid[r].ravel()
        if not v.any():
            per_roi.append((np.zeros(0, np.int32),
                            np.zeros((0, QUAD, NBINS), np.float32)))
            continue
        shp = valid[r].shape
        bc = lambda a: np.broadcast_to(a, shp).ravel()[v]
        sc = bc(scale[r]).astype(np.float32)
        bn = bc(bins[r]).astype(np.int64)
        cy0 = bc(y0i[r]).astype(np.int64)
        cy1 = bc(y1i[r]).astype(np.int64)
        cx0 = bc(x0i[r]).astype(np.int64)
        cx1 = bc(x1i[r]).astype(np.int64)
        ws = [bc(w00[r]) * sc, bc(w01[r]) * sc,
              bc(w10[r]) * sc, bc(w11[r]) * sc]
        cells = [cy0 * W + cx0, cy0 * W + cx1, cy1 * W + cx0, cy1 * W + cx1]

        cell_all = np.concatenate(cells)
        w_all = np.concatenate(ws).astype(np.float64)
        bin_all = np.concatenate([bn] * 4)

        quads = np.unique(cell_all >> 2).astype(np.int32)       # sorted
        qpos = np.searchsorted(quads, cell_all >> 2)
        key = (qpos * QUAD + (cell_all & 3)) * NBINS + bin_all
        S = np.bincount(key, weights=w_all,
                        minlength=len(quads) * QUAD * NBINS)
        S = S.astype(np.float32).reshape(len(quads), QUAD, NBINS)
        per_roi.append((quads + np.int32(b[r]) * np.int32(NQROWS), S))
    return per_roi


def _segments(per_roi):
    """Decompose each roi's sorted quad list into segments of <= KSEG
    consecutive quads. Returns per roi (seg_starts [m] int32,
    S8 [m, SEG_CELLS, NBINS] f32)."""
    out = []
    for quads, S in per_roi:
        if len(quads) == 0:
            out.append((np.zeros(0, np.int32),
                        np.zeros((0, SEG_CELLS, NBINS), np.float32)))
            continue
        run_bounds = np.where(np.diff(quads) != 1)[0] + 1
        starts_list = []
        s8_list = []
        for run_q, run_s in zip(np.split(quads, run_bounds),
                                np.split(S, run_bounds)):
            n = len(run_q)
            m = (n + KSEG - 1) // KSEG
            pad = m * KSEG - n
            starts_list.append(run_q[::KSEG])
            sp = np.concatenate(
                [run_s, np.zeros((pad, QUAD, NBINS), np.float32)], axis=0)
            s8_list.append(sp.reshape(m, SEG_CELLS, NBINS))
        out.append((np.concatenate(starts_list).astype(np.int32),
                    np.concatenate(s8_list, axis=0)))
    return out


def _balance(per_seg):
    """LPT greedy: assign rois to 8 cores balancing total segment count."""
    order = sorted(range(len(per_seg)),
                   key=lambda r: -len(per_seg[r][0]))
    loads = [0] * N_CORES
    core_rois = [[] for _ in range(N_CORES)]
    for rid in order:
        c = min(range(N_CORES), key=lambda i: loads[i])
        core_rois[c].append(rid)
        loads[c] += len(per_seg[rid][0])
    return core_rois


def _pack_core(rids, per_seg):
    """Pair rois (best-fit: minimize ceil((a+b)/CHUNK), then maximize fill)
    and place each pair chunk-aligned, so no chunk ever sees more than 2
    rois. Returns list of (rid, start_slot) and total slots used."""
    placements = [(rid, 0) for rid in rids if len(per_seg[rid][0]) == 0]
    items = [(len(per_seg[rid][0]), rid) for rid in rids
             if len(per_seg[rid][0]) > 0]
    items.sort(reverse=True)
    used = [False] * len(items)
    pairs = []
    for i, (a, rid_a) in enumerate(items):
        if used[i]:
            continue
        used[i] = True
        best_j, best_key = -1, None
        for j in range(len(items) - 1, i, -1):
            if used[j]:
                continue
            b = items[j][0]
            key = (-(-(a + b) // CHUNK), -(a + b))
            if best_key is None or key < best_key:
                best_key, best_j = key, j
        if best_j >= 0:
            used[best_j] = True
            pairs.append((rid_a, items[best_j][1]))
        else:
            pairs.append((rid_a, None))
    pos = 0
    for rid_a, rid_b in pairs:
        placements.append((rid_a, pos))
        sz = len(per_seg[rid_a][0])
        if rid_b is not None:
            placements.append((rid_b, pos + sz))
            sz += len(per_seg[rid_b][0])
        pos += -(-sz // CHUNK) * CHUNK
    return placements, pos


_PROGRAM_CACHE: dict = {}


def _build_program(C):
    """One SPMD Tile program for all 8 cores, parameterised by chunk count."""
    if C in _PROGRAM_CACHE:
        return _PROGRAM_CACHE[C]

    from concourse import bass, mybir, bacc
    from concourse.tile import TileContext

    nc = bacc.Bacc("TRN2", target_bir_lowering=False, debug=False,
                   num_devices=N_CORES)
    dataT = nc.declare_dram_parameter("dataT", [NROWS_ALL + KSEG, ROW_ELEMS],
                                      mybir.dt.bfloat16, isOutput=False)
    offs = nc.declare_dram_parameter("offs", [CHUNK, C],
                                     mybir.dt.int32, isOutput=False)
    spack = nc.declare_dram_parameter("spack", [CHUNK, C * SEG_CELLS * MBLK],
                                      mybir.dt.bfloat16, isOutput=False)
    out = nc.declare_dram_parameter("out", [MBLK, C * C_FEAT],
                                    mybir.dt.bfloat16, isOutput=True)

    with TileContext(nc) as tc:
        with (
            tc.tile_pool(name="const", bufs=1) as cpool,
            tc.tile_pool(name="gt", bufs=6) as gpool,
            tc.tile_pool(name="ps", bufs=6, space="PSUM") as pspool,
            tc.tile_pool(name="ob", bufs=3) as opool,
        ):
            offs_t = cpool.tile([CHUNK, C], mybir.dt.int32)
            nc.sync.dma_start(out=offs_t[:], in_=offs[:])
            s_t = cpool.tile([CHUNK, C * SEG_CELLS * MBLK], mybir.dt.bfloat16)
            # Load S per chunk-group so early matmuls can start sooner.
            scw = SEG_CELLS * MBLK
            for g in range(0, C, GOUT):
                hi = min(g + GOUT, C)
                nc.sync.dma_start(out=s_t[:, g * scw:hi * scw],
                                  in_=spack[:, g * scw:hi * scw])

            ob = None
            for k in range(C):
                gt = gpool.tile([CHUNK, SEG_ELEMS], mybir.dt.bfloat16)
                nc.gpsimd.indirect_dma_start(
                    out=gt[:],
                    out_offset=None,
                    in_=dataT[:],
                    in_offset=bass.IndirectOffsetOnAxis(
                        ap=offs_t[:, k:k + 1], axis=0),
                    bounds_check=NROWS_ALL - 1,
                    oob_is_err=False,
                )
                ps = pspool.tile([MBLK, C_FEAT], mybir.dt.float32)
                for e in range(SEG_CELLS):
                    nc.tensor.matmul(
                        ps[:],
                        lhsT=s_t[:, (k * SEG_CELLS + e) * MBLK:
                                 (k * SEG_CELLS + e + 1) * MBLK],
                        rhs=gt[:, e * C_FEAT:(e + 1) * C_FEAT],
                        start=(e == 0),
                        stop=(e == SEG_CELLS - 1),
                    )
                if k % GOUT == 0:
                    gw = min(GOUT, C - k)
                    ob = opool.tile([MBLK, gw * C_FEAT], mybir.dt.bfloat16)
                j = k % GOUT
                if k % 2 == 0:
                    nc.vector.tensor_copy(
                        out=ob[:, j * C_FEAT:(j + 1) * C_FEAT], in_=ps[:])
                else:
                    nc.scalar.copy(
                        out=ob[:, j * C_FEAT:(j + 1) * C_FEAT], in_=ps[:])
                if j == gw - 1:
                    k0 = k - j
                    nc.sync.dma_start(
                        out=out[:, k0 * C_FEAT:(k + 1) * C_FEAT], in_=ob[:])
    nc.compile()
    _PROGRAM_CACHE[C] = nc
    return nc


def _to_bf16(x):
    import ml_dtypes
    return x.astype(ml_dtypes.bfloat16)


def _core_inputs(per_seg, core_rois, C, dataT_b):
    in_maps = []
    chunk_maps = []          # per core: list over chunks of [rid_b0, rid_b1]
    import ml_dtypes
    for c in range(N_CORES):
        placements, _ = _pack_core(core_rois[c], per_seg)
        offs = np.full((CHUNK, C), OOB, np.int32)
        spack = np.zeros((CHUNK, C * SEG_CELLS * MBLK), np.float32)
        cmap = [[-1, -1] for _ in range(C)]
        nplaced = 0
        for rid, start in placements:
            starts, S8 = per_seg[rid]
            q = len(starts)
            if q == 0:
                continue
            beta = nplaced % 2
            nplaced += 1
            rows = np.arange(start, start + q)
            ck = rows // CHUNK
            pp = rows % CHUNK
            offs[pp, ck] = starts
            # scatter S8 into spack: col = (ck*SEG_CELLS + e)*MBLK + beta*49
            for e in range(SEG_CELLS):
                cols = (ck * SEG_CELLS + e) * MBLK + beta * NBINS
                spack[pp[:, None], cols[:, None] + np.arange(NBINS)[None, :]] = S8[:, e, :]
            for kk in np.unique(ck):
                cmap[kk][beta] = rid
        in_maps.append({
            "dataT": dataT_b,
            "offs": offs,
            "spack": spack.astype(ml_dtypes.bfloat16),
        })
        chunk_maps.append(cmap)
    return in_maps, chunk_maps


def kernel(data: np.ndarray, rois: np.ndarray, offset: np.ndarray) -> np.ndarray:
    from concourse.bass_utils import run_bass_kernel_spmd

    data = np.ascontiguousarray(data, dtype=np.float32)
    rois = np.asarray(rois, dtype=np.float32)
    offset = np.asarray(offset, dtype=np.float32)
    R = rois.shape[0]

    per_roi = _host_tables(rois, offset)
    per_seg = _segments(per_roi)
    core_rois = _balance(per_seg)
    C = 0
    for c in range(N_CORES):
        _, rows = _pack_core(core_rois[c], per_seg)
        C = max(C, (rows + CHUNK - 1) // CHUNK)
    nc = _build_program(C)

    # channel-last quad-row layout, both images stacked + KSEG zero pad rows
    dataT = np.concatenate([
        np.ascontiguousarray(data[i].transpose(1, 2, 0)).reshape(NQROWS, ROW_ELEMS)
        for i in range(N_IMG)
    ] + [np.zeros((KSEG, ROW_ELEMS), np.float32)], axis=0)
    dataT_b = _to_bf16(dataT)
    in_maps, chunk_maps = _core_inputs(per_seg, core_rois, C, dataT_b)

    res = run_bass_kernel_spmd(nc, in_maps, list(range(N_CORES)), trace=False)

    out_full = np.zeros((R, C_FEAT, P, P), np.float32)
    for c in range(N_CORES):
        o = np.asarray(res.results[c]["out"]).astype(np.float32)  # [98, C*256]
        o = o.reshape(MBLK, C, C_FEAT).transpose(1, 0, 2)         # [C, 98, 256]
        for k in range(C):
            for beta in range(2):
                rid = chunk_maps[c][k][beta]
                if rid >= 0:
                    blk = o[k, beta * NBINS:(beta + 1) * NBINS]   # [49, 256]
                    out_full[rid] += blk.T.reshape(C_FEAT, P, P)
    return out_full
